# revision 74
# baseline (speedup 1.0000x reference)
"""Multi-head causal attention (B=4, T=2048, D=1024, H=16) on 8 Trainium2 cores.

Sharding: core c = (b, g) with b = c//2 (batch), g = c%2 (head-group of 8 heads).
Each core: Q/K/V projections for its 8 heads (column-parallel), causal attention,
row-parallel partial output projection. Host sums the g=0/g=1 partials + bias.

v3 design (cost-model-driven):
  - Matmul cost in the timeline model = out-free-rows x cycle x cpr, independent
    of contraction depth/partitions. fp32r: cpr=1 only for N>=256; bf16: cpr=1
    at any N.
  - Scores (S^T layout, fp32r, N=512 spans), exp -> pt in bf16.
  - AV is FLIPPED to q-partition layout: per (q-tile 128, key-chunk) matmul with
    lhsT = pt chunk (keys x 128q, bf16), rhs = V chunk [V|1] (keys x 65, bf16),
    costing 65 rows instead of streaming 512 q columns: 143k -> 71k rows.
    AV runs QTILE-MAJOR (each (qt,u) accumulation contiguous) because psum
    allows only one open accumulation group per 2KB bank; pt chunks for the
    whole pair stay buffered in SBUF.
  - ctx comes out q-major; normalize by 1/Z (psum col 64) via per-partition
    TensorScalarPtr, then PE-transpose (128x128, bf16) back to ctxT layout for
    the row-parallel output projection (bf16 x bf16, N=512).
  - qT/ctxT are span-sliced pool tiles (only one span live); kT persists full.
  - proj(s+1) and outproj(s-1) groups are spread as PE fillers through
    attention(s) so PE never stalls on the exp (ACT) chain.
"""

import os
import sys

try:
    import concourse.bass  # noqa: F401
except ImportError:  # pragma: no cover
    sys.path.insert(0, "/opt/trn_rl_repo")

import numpy as np

B, T, D = 4, 2048, 1024
H, HD = 16, 64
NCORES = 8
NPAIR = 4       # head pairs per core
NSPAN = 4       # q spans of 512
SPAN = 512
NKC = 16        # key chunks of 128
KC = 128
NDC = 8         # D chunks of 128
P = 128

_CACHE = {}


def _build():
    import concourse.bacc as bacc
    import concourse.mybir as mybir
    import concourse.tile as tile

    f32 = mybir.dt.float32
    f32r = mybir.dt.float32r
    bf16 = mybir.dt.bfloat16
    u16 = mybir.dt.uint16
    Exp = mybir.ActivationFunctionType.Exp
    Copy = mybir.ActivationFunctionType.Copy

    dbg = bool(os.environ.get("KDEBUG"))
    nc = bacc.Bacc("TRN2", target_bir_lowering=False, debug=False,
                   num_devices=1 if dbg else NCORES)

    f16 = mybir.dt.float16
    xT_h = nc.dram_tensor("xT", (D, T), f16, kind="ExternalInput")
    wqT_h = nc.dram_tensor("wqT", (D, 512), f16, kind="ExternalInput")
    wkT_h = nc.dram_tensor("wkT", (D, 512), f16, kind="ExternalInput")
    wvT_h = nc.dram_tensor("wvT", (D, 512), f16, kind="ExternalInput")
    woT_h = nc.dram_tensor("woT", (512, D), u16, kind="ExternalInput")
    out_h = nc.dram_tensor("out", (T, D), f32, kind="ExternalOutput")
    if dbg:
        dbg_h = {
            "qT_o": nc.dram_tensor("qT_o", (NPAIR, P, SPAN), f32, kind="ExternalOutput"),
            "kT_o": nc.dram_tensor("kT_o", (NPAIR, P, T), f32, kind="ExternalOutput"),
            "vb_o": nc.dram_tensor("vb_o", (P, NKC, NPAIR, 2, HD + 1), bf16,
                                   kind="ExternalOutput"),
            "ctx_o": nc.dram_tensor("ctx_o", (NPAIR, P, T), bf16,
                                    kind="ExternalOutput"),
            "pt_o": nc.dram_tensor("pt_o", (4, P, 2, SPAN), bf16,
                                   kind="ExternalOutput"),
            "av_o": nc.dram_tensor("av_o", (2, P, 4, HD + 1), f32,
                                   kind="ExternalOutput"),
            "id_o": nc.dram_tensor("id_o", (P, P), bf16, kind="ExternalOutput"),
            "mask_o": nc.dram_tensor("mask_o", (P, KC), bf16,
                                     kind="ExternalOutput"),
        }

    xT_d = xT_h.ap().rearrange("(dc p) t -> p dc t", p=P)       # (128, 8, 2048)
    wq_d = wqT_h.ap().rearrange("(dc p) f -> p dc f", p=P)      # (128, 8, 512)
    wk_d = wkT_h.ap().rearrange("(dc p) f -> p dc f", p=P)
    wv_d = wvT_h.ap().rearrange("(dc p) f -> p dc f", p=P)
    wo_d = woT_h.ap().rearrange("(pc p) f -> p pc f", p=P)      # (128, 4, 1024)

    with tile.TileContext(nc) as tc:
        with (
            tc.tile_pool(name="persist", bufs=1) as persist,
            tc.tile_pool(name="xp", bufs=2) as xpool,
            tc.tile_pool(name="qp", bufs=2) as qpool,
            tc.tile_pool(name="cp", bufs=3) as cpool,
            tc.tile_pool(name="ptp", bufs=8) as ptpool,
            tc.tile_pool(name="sbc", bufs=6) as sbcpool,
            tc.tile_pool(name="rzp", bufs=6) as rzpool,
            tc.tile_pool(name="stg", bufs=4) as stgpool,
            tc.tile_pool(name="psS", bufs=2, space="PSUM") as psS,
            tc.tile_pool(name="psAV", bufs=2, space="PSUM") as psAV,
            tc.tile_pool(name="psT", bufs=2, space="PSUM") as psTr,
        ):
            kT = [persist.tile([P, T], f16, tag=f"kT{i}", name=f"kT{i}")
                  for i in range(NPAIR)]
            # [V | 1] per (key-chunk, pair, head): ones col -> Z in AV psum col 64
            Vb = persist.tile([P, NKC, NPAIR, 2, HD + 1], bf16, tag="Vb", name="Vb")
            wq = persist.tile([P, NDC, 512], f16, tag="wq", name="wq")
            wk = persist.tile([P, NDC, 512], f16, tag="wk", name="wk")
            wv = persist.tile([P, NDC, 512], f16, tag="wv", name="wv")
            wo = persist.tile([P, 4, D], u16, tag="wo", name="wo")
            mask01 = persist.tile([P, KC], bf16, tag="mask01", name="mask01")
            ident = persist.tile([P, P], bf16, tag="ident", name="ident")
            one = nc.const_aps.tensor(1.0, (P, 1))

            nc.vector.tensor_copy(
                Vb[:, :, :, :, HD:HD + 1], one.to_broadcast((P, NKC, NPAIR, 2, 1)))
            # causal diag mask: mask01[p, f] = 1.0 if p <= f else 0.0
            nc.gpsimd.memset(mask01[:], 1.0)
            nc.gpsimd.affine_select(
                out=mask01[:], in_=mask01[:],
                compare_op=mybir.AluOpType.is_ge, fill=0.0,
                base=0, channel_multiplier=-1, pattern=[[1, KC]],
            )
            # identity for PE transpose: keep p <= f, then keep p >= f
            nc.gpsimd.memset(ident[:], 1.0)
            nc.gpsimd.affine_select(
                out=ident[:], in_=ident[:],
                compare_op=mybir.AluOpType.is_ge, fill=0.0,
                base=0, channel_multiplier=-1, pattern=[[1, P]],
            )
            nc.gpsimd.affine_select(
                out=ident[:], in_=ident[:],
                compare_op=mybir.AluOpType.is_ge, fill=0.0,
                base=0, channel_multiplier=1, pattern=[[-1, P]],
            )

            # ---- initial DMAs: wq/x0 first (Q proj starts earliest), then
            # wk (attention needs kT early), wv, wo ----
            xts = {0: xpool.tile([P, NDC, SPAN], f16, tag="xt", name="xt0")}
            for q4 in range(4):
                nc.sync.dma_start(wq[:, 0, q4 * P:(q4 + 1) * P],
                                  wq_d[:, 0, q4 * P:(q4 + 1) * P])
                if q4 < 2:
                    nc.scalar.dma_start(
                        xts[0][:, 0, q4 * 256:(q4 + 1) * 256],
                        xT_d[:, 0, q4 * 256:(q4 + 1) * 256])
            # coarse DMAs after the fine first chunks: per-DMA fixed cost
            # (~1.2us issue+HWDGE) dominates over transfer at fp16 sizes
            nc.sync.dma_start(wq[:, 1:4], wq_d[:, 1:4])
            nc.scalar.dma_start(xts[0][:, 1:4], xT_d[:, 1:4, 0:SPAN])
            nc.sync.dma_start(wq[:, 4:8], wq_d[:, 4:8])
            nc.scalar.dma_start(xts[0][:, 4:8], xT_d[:, 4:8, 0:SPAN])
            nc.sync.dma_start(wk[:, 0:4], wk_d[:, 0:4])
            nc.scalar.dma_start(wv[:, 0:4], wv_d[:, 0:4])
            nc.sync.dma_start(wk[:, 4:8], wk_d[:, 4:8])
            nc.scalar.dma_start(wv[:, 4:8], wv_d[:, 4:8])
            nc.sync.dma_start(wo[:], wo_d[:])

            qts = {}   # (sp, pr) -> (P, SPAN) f32r tile
            cts = {}   # (sp, pr) -> (P, SPAN) bf16 tile

            # ---------- emission helpers ----------
            def proj_qk(w, pr, sp, xt, scale, isq):
                def emit():
                    ps = psTr.tile([P, SPAN], f32, tag="tr", name="psqk")
                    for dc in range(NDC):
                        nc.tensor.matmul(
                            ps[:], w[:, dc, pr * P:(pr + 1) * P], xt[:, dc],
                            start=(dc == 0), stop=(dc == NDC - 1))
                    if isq:
                        dest = qpool.tile([P, SPAN], f16, tag=f"qT{pr}",
                                          name=f"qT{pr}_{sp}")
                        qts[(sp, pr)] = dest
                        nc.vector.tensor_scalar_mul(dest[:], ps[:], scale)
                    else:
                        nc.vector.tensor_scalar_mul(
                            kT[pr][:, sp * SPAN:(sp + 1) * SPAN], ps[:], scale)
                return emit

            def proj_v(sp, tb, xt):
                def emit():
                    ps = psTr.tile([P, SPAN], f32, tag="tr", name="psv")
                    for dc in range(NDC):
                        nc.tensor.matmul(
                            ps[:], xt[:, dc, tb * P:(tb + 1) * P], wv[:, dc],
                            start=(dc == 0), stop=(dc == NDC - 1))
                    kc = sp * 4 + tb
                    psv = ps[:].rearrange("p (pr u f) -> p pr u f", u=2, f=HD)
                    nc.vector.tensor_copy(Vb[:, kc, :, :, 0:HD], psv)
                return emit

            def proj_groups(sp, xt):
                gs = [proj_qk(wq, 0, sp, xt, 0.125, True),
                      proj_qk(wk, 0, sp, xt, 1.0, False)]
                gs += [proj_v(sp, tb, xt) for tb in range(4)]
                for pr in range(1, NPAIR):
                    gs.append(proj_qk(wq, pr, sp, xt, 0.125, True))
                    gs.append(proj_qk(wk, pr, sp, xt, 1.0, False))
                return gs

            def outproj_group(sp, tb, os_, alt_pool=False, act_copy=False,
                              alt_dma=False):
                def emit():
                    if alt_pool:
                        ps = psS.tile([P, 2, SPAN], f32, tag="psS",
                                      name="pso2")[:, 0, :]
                    else:
                        ps = psTr.tile([P, SPAN], f32, tag="tr", name="pso")[:]
                    for pc in range(NPAIR):
                        nc.tensor.matmul(
                            ps,
                            cts[(sp, pc)][:, (tb - sp * 4) * P:(tb - sp * 4 + 1) * P],
                            wo[:, pc, os_ * SPAN:(os_ + 1) * SPAN].bitcast(bf16),
                            start=(pc == 0), stop=(pc == NPAIR - 1))
                    stage = stgpool.tile([P, SPAN], f32, tag="st", name="stage")
                    if act_copy:
                        nc.scalar.activation(stage[:], ps, Copy)
                    else:
                        nc.vector.tensor_copy(stage[:], ps)
                    dma_q = nc.scalar if alt_dma else nc.sync
                    dma_q.dma_start(
                        out_h.ap()[tb * P:(tb + 1) * P,
                                   os_ * SPAN:(os_ + 1) * SPAN], stage[:])
                return emit

            def outproj_groups(sp):
                return [outproj_group(sp, tb, os_)
                        for tb in range(sp * 4, (sp + 1) * 4) for os_ in range(2)]

            # ---------- attention for one span ----------
            LAG = 3

            def attn_span(s, fillers):
                K = 4 * (s + 1)
                nslot = (K + LAG + 2) * NPAIR
                state = {"slot": 0, "fi": 0}

                def pace():
                    # hold back a few fillers for the span boundary, where
                    # the next pair-0 exp warm-up (or the span-3 tail) has
                    # no native PE work
                    tgt = min(max(0, len(fillers) - 3),
                              len(fillers) * (state["slot"] + 1) // nslot)
                    while state["fi"] < tgt:
                        fillers[state["fi"]]()
                        state["fi"] += 1

                def tick():
                    state["slot"] += 1
                    pace()

                for pr in range(NPAIR):
                    # 2 qtiles packed per bank; accumulation via start=False
                    # onto memset-zeroed psum (one open group per bank is a
                    # hw constraint only for start=True zero-region resets)
                    av = [psAV.tile([P, 2, 2, HD + 1], f32, tag="av",
                                    name=f"av{j}") for j in range(2)]
                    for j in range(2):
                        nc.vector.memset(av[j][:], 0.0)
                    ct = cpool.tile([P, SPAN], bf16, tag=f"cT{pr}",
                                    name=f"cT{pr}_{s}")
                    cts[(s, pr)] = ct
                    pts = {}
                    deferred = []
                    pend = []
                    qt_tile = qts[(s, pr)]

                    def emit_qk(kj, qt_tile=qt_tile, pr=pr, pts=pts):
                        m = kj - 4 * s
                        sl0 = 0 if m < 0 else m * KC
                        c0 = 0 if m < 0 else m * KC
                        ss = psS.tile([P, 2, SPAN], f32, tag="psS", name="ss")
                        pt = ptpool.tile([P, 2, SPAN], bf16, tag="pt", name="pt")
                        for u in range(2):
                            lo, hi = u * HD, (u + 1) * HD
                            nc.tensor.matmul(
                                ss[:, u, sl0:],
                                kT[pr][lo:hi, kj * KC:(kj + 1) * KC],
                                qt_tile[lo:hi, sl0:],
                                start=True, stop=True)
                        nc.scalar.activation(pt[:, :, c0:], ss[:, :, c0:], Exp)
                        if m >= 0:
                            nc.vector.tensor_mul(
                                pt[:, :, c0:c0 + KC], pt[:, :, c0:c0 + KC],
                                mask01[:].rearrange("p (u f) -> p u f", u=1)
                                .to_broadcast((P, 2, KC)))
                        if dbg and s == 0 and pr == 0:
                            nc.sync.dma_start(dbg_h["pt_o"].ap()[kj], pt[:])
                        pts[kj] = pt

                    def evict(qt, av=av, pr=pr, ct=ct):
                        j, qtl = qt // 2, qt % 2
                        if dbg and s == 0 and pr == 0 and qt in (1, 3):
                            avs = stgpool.tile([P, 2 * 2 * (HD + 1)], f32,
                                               tag="st", name="avs")
                            nc.vector.tensor_copy(
                                avs[:].rearrange("p (a u f) -> p a u f",
                                                 a=2, f=HD + 1), av[j][:])
                            nc.sync.dma_start(dbg_h["av_o"].ap()[j], avs[:])
                        rz = rzpool.tile([P, 2], f32, tag="rz", name="rz")
                        sbc = sbcpool.tile([P, 2, HD], bf16, tag="sbc", name="sbc")
                        for u in range(2):
                            nc.vector.reciprocal(
                                rz[:, u:u + 1], av[j][:, qtl, u, HD:HD + 1])
                            nc.vector.tensor_scalar_mul(
                                sbc[:, u, :], av[j][:, qtl, u, 0:HD],
                                rz[:, u:u + 1])

                        def fin():
                            psx = psTr.tile([P, P], bf16, tag="tr", name="pst")
                            nc.tensor.transpose(
                                psx[:], sbc[:].rearrange("p u f -> p (u f)"),
                                ident[:])
                            nc.vector.tensor_copy(
                                ct[:, qt * P:(qt + 1) * P], psx[:])
                        deferred.append(fin)

                    def emit_av(kj, av=av, pr=pr, pts=pts):
                        m = kj - 4 * s
                        pt = pts.pop(kj)
                        for mq in range(max(0, m), 4):
                            qi = 4 * s + mq
                            j, qtl = mq // 2, mq % 2
                            for u in range(2):
                                nc.tensor.matmul(
                                    av[j][:, qtl, u, :],
                                    pt[:, u, mq * KC:(mq + 1) * KC],
                                    Vb[:, kj, pr, u, :],
                                    start=False, stop=(kj == qi),
                                    skip_group_check=True)
                        if m >= 0:
                            evict(m)

                    for kj in range(K):
                        emit_qk(kj)
                        pend.append(kj)
                        if len(pend) > LAG:
                            emit_av(pend.pop(0))
                        if len(deferred) > 3:
                            deferred.pop(0)()
                        tick()
                    while pend:
                        emit_av(pend.pop(0))
                        if len(deferred) > 3 or (not pend and deferred):
                            deferred.pop(0)()
                        tick()
                    while deferred:
                        deferred.pop(0)()
                        tick()
                # flush remaining fillers
                while state["fi"] < len(fillers):
                    fillers[state["fi"]]()
                    state["fi"] += 1

            # ---------- main schedule ----------
            # span-0 projections dc-major so matmul consumption paces with
            # chunkwise DMA arrival; 4 accumulators (2 psTr + 2 psS banks,
            # free at startup). First matmuls run half-N on the split first
            # chunks; h==0's start=True zeroes the whole psum zero-region so
            # h==1 accumulates with start=False.
            def proj0():
                def accs4():
                    a = [psTr.tile([P, SPAN], f32, tag="tr", name="p0")[:]
                         for _ in range(2)]
                    a += [psS.tile([P, 2, SPAN], f32, tag="psS",
                                   name="p0s")[:, 0, :] for _ in range(2)]
                    return a
                xt = xts[0]
                qa = accs4()
                for h in range(2):
                    for pr in range(NPAIR):
                        nc.tensor.matmul(
                            qa[pr][:, h * 256:(h + 1) * 256],
                            wq[:, 0, pr * P:(pr + 1) * P],
                            xt[:, 0, h * 256:(h + 1) * 256],
                            start=(h == 0), stop=False, skip_group_check=True)
                for dc in range(1, NDC):
                    for pr in range(NPAIR):
                        nc.tensor.matmul(
                            qa[pr], wq[:, dc, pr * P:(pr + 1) * P], xt[:, dc],
                            start=False, stop=(dc == NDC - 1),
                            skip_group_check=True)
                for pr in range(NPAIR):
                    dest = qpool.tile([P, SPAN], f16, tag=f"qT{pr}",
                                      name=f"qT{pr}_0")
                    qts[(0, pr)] = dest
                    nc.vector.tensor_scalar_mul(dest[:], qa[pr], 0.125)
                ka = accs4()
                for dc in range(NDC):
                    for pr in range(NPAIR):
                        nc.tensor.matmul(
                            ka[pr], wk[:, dc, pr * P:(pr + 1) * P], xt[:, dc],
                            start=(dc == 0), stop=(dc == NDC - 1))
                for pr in range(NPAIR):
                    nc.vector.tensor_scalar_mul(kT[pr][:, 0:SPAN], ka[pr], 1.0)
                va = accs4()
                for dc in range(NDC):
                    for tb in range(4):
                        nc.tensor.matmul(
                            va[tb], xt[:, dc, tb * P:(tb + 1) * P], wv[:, dc],
                            start=(dc == 0), stop=(dc == NDC - 1))
                for tb in range(4):
                    psv = va[tb].rearrange("p (pr u f) -> p pr u f", u=2, f=HD)
                    nc.vector.tensor_copy(Vb[:, tb, :, :, 0:HD], psv)

            proj0()
            carry = []
            for s in range(NSPAN):
                fillers = []
                if s + 1 < NSPAN:
                    xt = xpool.tile([P, NDC, SPAN], f16, tag="xt",
                                    name=f"xt{s + 1}")
                    xts[s + 1] = xt
                    for hdc in range(2):
                        nc.sync.dma_start(
                            xt[:, hdc * 4:(hdc + 1) * 4],
                            xT_d[:, hdc * 4:(hdc + 1) * 4,
                                 (s + 1) * SPAN:(s + 2) * SPAN])
                    pg = proj_groups(s + 1, xt)
                    if s >= 1:
                        # defer Q2/K2/Q3/K3 of the next span into that
                        # span's own filler stream (early spans have filler
                        # surplus; later spans are exp-bound and starve)
                        fillers += pg[:8]
                        carry = pg[8:]
                    else:
                        fillers += pg
                if s == 1:
                    fillers += outproj_groups(0)
                elif s == 3:
                    # span 3's own attention is ACT-bound: reserve two
                    # spans' outproj work as filler here (cpool bufs=3
                    # removes the ctxT-slot recycle deadline for span 1)
                    fillers += outproj_groups(1) + outproj_groups(2)
                attn_span(s, fillers)
            for gi in range(8):
                tb, os_ = 12 + gi // 2, gi % 2
                outproj_group(3, tb, os_, act_copy=True,
                              alt_dma=(os_ == 1))()
            if dbg:
                for i in range(NPAIR):
                    nc.sync.dma_start(dbg_h["qT_o"].ap()[i],
                                      qts[(0, i)][:].bitcast(f32))
                    nc.sync.dma_start(dbg_h["kT_o"].ap()[i], kT[i][:].bitcast(f32))
                    nc.sync.dma_start(dbg_h["ctx_o"].ap()[i][:, 3 * SPAN:],
                                      cts[(3, i)][:])
                nc.sync.dma_start(dbg_h["vb_o"].ap()[:], Vb[:])
                nc.sync.dma_start(dbg_h["id_o"].ap()[:], ident[:])
                nc.sync.dma_start(dbg_h["mask_o"].ap()[:], mask01[:])

    nc.compile()
    return nc


def get_nc():
    if "nc" not in _CACHE:
        _CACHE["nc"] = _build()
    return _CACHE["nc"]


def kernel(x, Wq, Wk, Wv, Wo, bo):
    import ml_dtypes
    from concourse import bass_utils

    x = np.asarray(x, dtype=np.float32)
    Wq, Wk, Wv = (np.asarray(w, dtype=np.float32) for w in (Wq, Wk, Wv))
    Wo = np.asarray(Wo, dtype=np.float32)
    bo = np.asarray(bo, dtype=np.float32)

    in_maps = []
    for c in range(NCORES):
        b, g = c // 2, c % 2
        gsl = slice(g * 512, (g + 1) * 512)
        in_maps.append({
            "xT": np.ascontiguousarray(x[b].T).astype(np.float16),
            "wqT": np.ascontiguousarray(Wq[gsl].T).astype(np.float16),
            "wkT": np.ascontiguousarray(Wk[gsl].T).astype(np.float16),
            "wvT": np.ascontiguousarray(Wv[gsl].T).astype(np.float16),
            "woT": np.ascontiguousarray(Wo[:, gsl].T)
            .astype(ml_dtypes.bfloat16).view(np.uint16),
        })

    nc = get_nc()
    res = bass_utils.run_bass_kernel_spmd(nc, in_maps, core_ids=list(range(NCORES)))
    parts = [res.results[c]["out"] for c in range(NCORES)]
    out = np.stack([parts[2 * b] + parts[2 * b + 1] + bo for b in range(B)])
    return out.astype(np.float32)


# revision 75
# speedup vs baseline: 1.0299x; 1.0299x over previous
"""Multi-head causal attention (B=4, T=2048, D=1024, H=16) on 8 Trainium2 cores.

Sharding: core c = (b, g) with b = c//2 (batch), g = c%2 (head-group of 8 heads).
Each core: Q/K/V projections for its 8 heads (column-parallel), causal attention,
row-parallel partial output projection. Host sums the g=0/g=1 partials + bias.

v3 design (cost-model-driven):
  - Matmul cost in the timeline model = out-free-rows x cycle x cpr, independent
    of contraction depth/partitions. fp32r: cpr=1 only for N>=256; bf16: cpr=1
    at any N.
  - Scores (S^T layout, fp32r, N=512 spans), exp -> pt in bf16.
  - AV is FLIPPED to q-partition layout: per (q-tile 128, key-chunk) matmul with
    lhsT = pt chunk (keys x 128q, bf16), rhs = V chunk [V|1] (keys x 65, bf16),
    costing 65 rows instead of streaming 512 q columns: 143k -> 71k rows.
    AV runs QTILE-MAJOR (each (qt,u) accumulation contiguous) because psum
    allows only one open accumulation group per 2KB bank; pt chunks for the
    whole pair stay buffered in SBUF.
  - ctx comes out q-major; normalize by 1/Z (psum col 64) via per-partition
    TensorScalarPtr, then PE-transpose (128x128, bf16) back to ctxT layout for
    the row-parallel output projection (bf16 x bf16, N=512).
  - qT/ctxT are span-sliced pool tiles (only one span live); kT persists full.
  - proj(s+1) and outproj(s-1) groups are spread as PE fillers through
    attention(s) so PE never stalls on the exp (ACT) chain.
"""

import os
import sys

try:
    import concourse.bass  # noqa: F401
except ImportError:  # pragma: no cover
    sys.path.insert(0, "/opt/trn_rl_repo")

import numpy as np

B, T, D = 4, 2048, 1024
H, HD = 16, 64
NCORES = 8
NPAIR = 4       # head pairs per core
NSPAN = 4       # q spans of 512
SPAN = 512
NKC = 16        # key chunks of 128
KC = 128
NDC = 8         # D chunks of 128
P = 128

_CACHE = {}


def _build():
    import concourse.bacc as bacc
    import concourse.mybir as mybir
    import concourse.tile as tile

    f32 = mybir.dt.float32
    f32r = mybir.dt.float32r
    bf16 = mybir.dt.bfloat16
    u16 = mybir.dt.uint16
    Exp = mybir.ActivationFunctionType.Exp
    Copy = mybir.ActivationFunctionType.Copy

    dbg = bool(os.environ.get("KDEBUG"))
    nc = bacc.Bacc("TRN2", target_bir_lowering=False, debug=False,
                   num_devices=1 if dbg else NCORES)

    f16 = mybir.dt.float16
    xT_h = nc.dram_tensor("xT", (D, T), f16, kind="ExternalInput")
    wqT_h = nc.dram_tensor("wqT", (D, 512), f16, kind="ExternalInput")
    wkT_h = nc.dram_tensor("wkT", (D, 512), f16, kind="ExternalInput")
    wvT_h = nc.dram_tensor("wvT", (D, 512), f16, kind="ExternalInput")
    woT_h = nc.dram_tensor("woT", (512, D), u16, kind="ExternalInput")
    out_h = nc.dram_tensor("out", (T, D), f32, kind="ExternalOutput")
    if dbg:
        dbg_h = {
            "qT_o": nc.dram_tensor("qT_o", (NPAIR, P, SPAN), f32, kind="ExternalOutput"),
            "kT_o": nc.dram_tensor("kT_o", (NPAIR, P, T), f32, kind="ExternalOutput"),
            "vb_o": nc.dram_tensor("vb_o", (P, NKC, NPAIR, 2, HD + 1), bf16,
                                   kind="ExternalOutput"),
            "ctx_o": nc.dram_tensor("ctx_o", (NPAIR, P, T), bf16,
                                    kind="ExternalOutput"),
            "pt_o": nc.dram_tensor("pt_o", (4, P, 2, SPAN), bf16,
                                   kind="ExternalOutput"),
            "av_o": nc.dram_tensor("av_o", (2, P, 4, HD + 1), f32,
                                   kind="ExternalOutput"),
            "id_o": nc.dram_tensor("id_o", (P, P), bf16, kind="ExternalOutput"),
            "mask_o": nc.dram_tensor("mask_o", (P, KC), bf16,
                                     kind="ExternalOutput"),
        }

    xT_d = xT_h.ap().rearrange("(dc p) t -> p dc t", p=P)       # (128, 8, 2048)
    wq_d = wqT_h.ap().rearrange("(dc p) f -> p dc f", p=P)      # (128, 8, 512)
    wk_d = wkT_h.ap().rearrange("(dc p) f -> p dc f", p=P)
    wv_d = wvT_h.ap().rearrange("(dc p) f -> p dc f", p=P)
    wo_d = woT_h.ap().rearrange("(pc p) f -> p pc f", p=P)      # (128, 4, 1024)

    with tile.TileContext(nc) as tc:
        with (
            tc.tile_pool(name="persist", bufs=1) as persist,
            tc.tile_pool(name="xp", bufs=2) as xpool,
            tc.tile_pool(name="qp", bufs=2) as qpool,
            tc.tile_pool(name="cp", bufs=3) as cpool,
            tc.tile_pool(name="ptp", bufs=8) as ptpool,
            tc.tile_pool(name="sbc", bufs=6) as sbcpool,
            tc.tile_pool(name="rzp", bufs=6) as rzpool,
            tc.tile_pool(name="stg", bufs=4) as stgpool,
            tc.tile_pool(name="psS", bufs=2, space="PSUM") as psS,
            tc.tile_pool(name="psAV", bufs=2, space="PSUM") as psAV,
            tc.tile_pool(name="psT", bufs=2, space="PSUM") as psTr,
        ):
            kT = [persist.tile([P, T], f16, tag=f"kT{i}", name=f"kT{i}")
                  for i in range(NPAIR)]
            # [V | 1] per (key-chunk, pair, head): ones col -> Z in AV psum col 64
            Vb = persist.tile([P, NKC, NPAIR, 2, HD + 1], bf16, tag="Vb", name="Vb")
            wq = persist.tile([P, NDC, 512], f16, tag="wq", name="wq")
            wk = persist.tile([P, NDC, 512], f16, tag="wk", name="wk")
            wv = persist.tile([P, NDC, 512], f16, tag="wv", name="wv")
            wo = persist.tile([P, 4, D], u16, tag="wo", name="wo")
            mask01 = persist.tile([P, KC], bf16, tag="mask01", name="mask01")
            ident = persist.tile([P, P], bf16, tag="ident", name="ident")
            one = nc.const_aps.tensor(1.0, (P, 1))

            nc.vector.tensor_copy(
                Vb[:, :, :, :, HD:HD + 1], one.to_broadcast((P, NKC, NPAIR, 2, 1)))
            # causal diag mask: mask01[p, f] = 1.0 if p <= f else 0.0
            nc.gpsimd.memset(mask01[:], 1.0)
            nc.gpsimd.affine_select(
                out=mask01[:], in_=mask01[:],
                compare_op=mybir.AluOpType.is_ge, fill=0.0,
                base=0, channel_multiplier=-1, pattern=[[1, KC]],
            )
            # identity for PE transpose: keep p <= f, then keep p >= f
            nc.gpsimd.memset(ident[:], 1.0)
            nc.gpsimd.affine_select(
                out=ident[:], in_=ident[:],
                compare_op=mybir.AluOpType.is_ge, fill=0.0,
                base=0, channel_multiplier=-1, pattern=[[1, P]],
            )
            nc.gpsimd.affine_select(
                out=ident[:], in_=ident[:],
                compare_op=mybir.AluOpType.is_ge, fill=0.0,
                base=0, channel_multiplier=1, pattern=[[-1, P]],
            )

            # ---- initial DMAs: wq/x0 first (Q proj starts earliest), then
            # wk (attention needs kT early), wv, wo ----
            xts = {0: xpool.tile([P, NDC, SPAN], f16, tag="xt", name="xt0")}
            for q4 in range(4):
                nc.sync.dma_start(wq[:, 0, q4 * P:(q4 + 1) * P],
                                  wq_d[:, 0, q4 * P:(q4 + 1) * P])
                if q4 < 2:
                    nc.scalar.dma_start(
                        xts[0][:, 0, q4 * 256:(q4 + 1) * 256],
                        xT_d[:, 0, q4 * 256:(q4 + 1) * 256])
            # coarse DMAs after the fine first chunks: per-DMA fixed cost
            # (~1.2us issue+HWDGE) dominates over transfer at fp16 sizes
            nc.sync.dma_start(wq[:, 1:4], wq_d[:, 1:4])
            nc.scalar.dma_start(xts[0][:, 1:4], xT_d[:, 1:4, 0:SPAN])
            nc.sync.dma_start(wq[:, 4:8], wq_d[:, 4:8])
            nc.scalar.dma_start(xts[0][:, 4:8], xT_d[:, 4:8, 0:SPAN])
            nc.sync.dma_start(wk[:, 0:4], wk_d[:, 0:4])
            nc.scalar.dma_start(wv[:, 0:4], wv_d[:, 0:4])
            nc.sync.dma_start(wk[:, 4:8], wk_d[:, 4:8])
            nc.scalar.dma_start(wv[:, 4:8], wv_d[:, 4:8])
            nc.sync.dma_start(wo[:], wo_d[:])

            qts = {}   # (sp, pr) -> (P, SPAN) f32r tile
            cts = {}   # (sp, pr) -> (P, SPAN) bf16 tile

            # ---------- emission helpers ----------
            def proj_qk(w, pr, sp, xt, scale, isq):
                def emit():
                    ps = psTr.tile([P, SPAN], f32, tag="tr", name="psqk")
                    for dc in range(NDC):
                        nc.tensor.matmul(
                            ps[:], w[:, dc, pr * P:(pr + 1) * P], xt[:, dc],
                            start=(dc == 0), stop=(dc == NDC - 1))
                    if isq:
                        dest = qpool.tile([P, SPAN], f16, tag=f"qT{pr}",
                                          name=f"qT{pr}_{sp}")
                        qts[(sp, pr)] = dest
                        nc.vector.tensor_scalar_mul(dest[:], ps[:], scale)
                    else:
                        nc.vector.tensor_scalar_mul(
                            kT[pr][:, sp * SPAN:(sp + 1) * SPAN], ps[:], scale)
                return emit

            def proj_v(sp, tb, xt):
                def emit():
                    ps = psTr.tile([P, SPAN], f32, tag="tr", name="psv")
                    for dc in range(NDC):
                        nc.tensor.matmul(
                            ps[:], xt[:, dc, tb * P:(tb + 1) * P], wv[:, dc],
                            start=(dc == 0), stop=(dc == NDC - 1))
                    kc = sp * 4 + tb
                    psv = ps[:].rearrange("p (pr u f) -> p pr u f", u=2, f=HD)
                    nc.vector.tensor_copy(Vb[:, kc, :, :, 0:HD], psv)
                return emit

            def proj_groups(sp, xt):
                gs = [proj_qk(wq, 0, sp, xt, 0.125, True),
                      proj_qk(wk, 0, sp, xt, 1.0, False)]
                gs += [proj_v(sp, tb, xt) for tb in range(4)]
                for pr in range(1, NPAIR):
                    gs.append(proj_qk(wq, pr, sp, xt, 0.125, True))
                    gs.append(proj_qk(wk, pr, sp, xt, 1.0, False))
                return gs

            def outproj_group(sp, tb, os_, alt_pool=False, act_copy=False,
                              alt_dma=False):
                def emit():
                    if alt_pool:
                        ps = psS.tile([P, 2, SPAN], f32, tag="psS",
                                      name="pso2")[:, 0, :]
                    else:
                        ps = psTr.tile([P, SPAN], f32, tag="tr", name="pso")[:]
                    for pc in range(NPAIR):
                        nc.tensor.matmul(
                            ps,
                            cts[(sp, pc)][:, (tb - sp * 4) * P:(tb - sp * 4 + 1) * P],
                            wo[:, pc, os_ * SPAN:(os_ + 1) * SPAN].bitcast(bf16),
                            start=(pc == 0), stop=(pc == NPAIR - 1))
                    stage = stgpool.tile([P, SPAN], f32, tag="st", name="stage")
                    if act_copy:
                        nc.scalar.activation(stage[:], ps, Copy)
                    else:
                        nc.vector.tensor_copy(stage[:], ps)
                    dma_q = nc.scalar if alt_dma else nc.sync
                    dma_q.dma_start(
                        out_h.ap()[tb * P:(tb + 1) * P,
                                   os_ * SPAN:(os_ + 1) * SPAN], stage[:])
                return emit

            def outproj_groups(sp):
                return [outproj_group(sp, tb, os_)
                        for tb in range(sp * 4, (sp + 1) * 4) for os_ in range(2)]

            # ---------- attention for one span ----------
            LAG = 3

            def attn_span(s, fillers):
                K = 4 * (s + 1)
                nslot = (K + LAG + 2) * NPAIR
                state = {"slot": 0, "fi": 0}

                def pace():
                    tgt = min(len(fillers),
                              len(fillers) * (state["slot"] + 1) // nslot)
                    while state["fi"] < tgt:
                        fillers[state["fi"]]()
                        state["fi"] += 1

                def tick():
                    state["slot"] += 1
                    pace()

                for pr in range(NPAIR):
                    # 2 qtiles packed per bank; accumulation via start=False
                    # onto memset-zeroed psum (one open group per bank is a
                    # hw constraint only for start=True zero-region resets)
                    av = [psAV.tile([P, 2, 2, HD + 1], f32, tag="av",
                                    name=f"av{j}") for j in range(2)]
                    for j in range(2):
                        nc.vector.memset(av[j][:], 0.0)
                    ct = cpool.tile([P, SPAN], bf16, tag=f"cT{pr}",
                                    name=f"cT{pr}_{s}")
                    cts[(s, pr)] = ct
                    pts = {}
                    deferred = []
                    pend = []
                    qt_tile = qts[(s, pr)]

                    def emit_qk(kj, qt_tile=qt_tile, pr=pr, pts=pts):
                        m = kj - 4 * s
                        sl0 = 0 if m < 0 else m * KC
                        c0 = 0 if m < 0 else m * KC
                        ss = psS.tile([P, 2, SPAN], f32, tag="psS", name="ss")
                        pt = ptpool.tile([P, 2, SPAN], bf16, tag="pt", name="pt")
                        for u in range(2):
                            lo, hi = u * HD, (u + 1) * HD
                            nc.tensor.matmul(
                                ss[:, u, sl0:],
                                kT[pr][lo:hi, kj * KC:(kj + 1) * KC],
                                qt_tile[lo:hi, sl0:],
                                start=True, stop=True)
                        nc.scalar.activation(pt[:, :, c0:], ss[:, :, c0:], Exp)
                        if m >= 0:
                            nc.vector.tensor_mul(
                                pt[:, :, c0:c0 + KC], pt[:, :, c0:c0 + KC],
                                mask01[:].rearrange("p (u f) -> p u f", u=1)
                                .to_broadcast((P, 2, KC)))
                        if dbg and s == 0 and pr == 0:
                            nc.sync.dma_start(dbg_h["pt_o"].ap()[kj], pt[:])
                        pts[kj] = pt

                    def evict(qt, av=av, pr=pr, ct=ct):
                        j, qtl = qt // 2, qt % 2
                        if dbg and s == 0 and pr == 0 and qt in (1, 3):
                            avs = stgpool.tile([P, 2 * 2 * (HD + 1)], f32,
                                               tag="st", name="avs")
                            nc.vector.tensor_copy(
                                avs[:].rearrange("p (a u f) -> p a u f",
                                                 a=2, f=HD + 1), av[j][:])
                            nc.sync.dma_start(dbg_h["av_o"].ap()[j], avs[:])
                        rz = rzpool.tile([P, 2], f32, tag="rz", name="rz")
                        sbc = sbcpool.tile([P, 2, HD], bf16, tag="sbc", name="sbc")
                        for u in range(2):
                            nc.vector.reciprocal(
                                rz[:, u:u + 1], av[j][:, qtl, u, HD:HD + 1])
                            nc.vector.tensor_scalar_mul(
                                sbc[:, u, :], av[j][:, qtl, u, 0:HD],
                                rz[:, u:u + 1])

                        def fin():
                            psx = psTr.tile([P, P], bf16, tag="tr", name="pst")
                            nc.tensor.transpose(
                                psx[:], sbc[:].rearrange("p u f -> p (u f)"),
                                ident[:])
                            nc.vector.tensor_copy(
                                ct[:, qt * P:(qt + 1) * P], psx[:])
                        deferred.append(fin)

                    def emit_av(kj, av=av, pr=pr, pts=pts):
                        m = kj - 4 * s
                        pt = pts.pop(kj)
                        for mq in range(max(0, m), 4):
                            qi = 4 * s + mq
                            j, qtl = mq // 2, mq % 2
                            for u in range(2):
                                nc.tensor.matmul(
                                    av[j][:, qtl, u, :],
                                    pt[:, u, mq * KC:(mq + 1) * KC],
                                    Vb[:, kj, pr, u, :],
                                    start=False, stop=(kj == qi),
                                    skip_group_check=True)
                        if m >= 0:
                            evict(m)

                    for kj in range(K):
                        emit_qk(kj)
                        pend.append(kj)
                        if len(pend) > LAG:
                            emit_av(pend.pop(0))
                        if len(deferred) > 3:
                            deferred.pop(0)()
                        tick()
                    while pend:
                        emit_av(pend.pop(0))
                        if len(deferred) > 3 or (not pend and deferred):
                            deferred.pop(0)()
                        tick()
                    while deferred:
                        deferred.pop(0)()
                        tick()
                # flush remaining fillers
                while state["fi"] < len(fillers):
                    fillers[state["fi"]]()
                    state["fi"] += 1

            # ---------- main schedule ----------
            # span-0 projections dc-major so matmul consumption paces with
            # chunkwise DMA arrival; 4 accumulators (2 psTr + 2 psS banks,
            # free at startup). First matmuls run half-N on the split first
            # chunks; h==0's start=True zeroes the whole psum zero-region so
            # h==1 accumulates with start=False.
            def proj0():
                def accs4():
                    a = [psTr.tile([P, SPAN], f32, tag="tr", name="p0")[:]
                         for _ in range(2)]
                    a += [psS.tile([P, 2, SPAN], f32, tag="psS",
                                   name="p0s")[:, 0, :] for _ in range(2)]
                    return a
                xt = xts[0]
                qa = accs4()
                for h in range(2):
                    for pr in range(NPAIR):
                        nc.tensor.matmul(
                            qa[pr][:, h * 256:(h + 1) * 256],
                            wq[:, 0, pr * P:(pr + 1) * P],
                            xt[:, 0, h * 256:(h + 1) * 256],
                            start=(h == 0), stop=False, skip_group_check=True)
                for dc in range(1, NDC):
                    for pr in range(NPAIR):
                        nc.tensor.matmul(
                            qa[pr], wq[:, dc, pr * P:(pr + 1) * P], xt[:, dc],
                            start=False, stop=(dc == NDC - 1),
                            skip_group_check=True)
                for pr in range(NPAIR):
                    dest = qpool.tile([P, SPAN], f16, tag=f"qT{pr}",
                                      name=f"qT{pr}_0")
                    qts[(0, pr)] = dest
                    nc.vector.tensor_scalar_mul(dest[:], qa[pr], 0.125)
                ka = accs4()
                for dc in range(NDC):
                    for pr in range(NPAIR):
                        nc.tensor.matmul(
                            ka[pr], wk[:, dc, pr * P:(pr + 1) * P], xt[:, dc],
                            start=(dc == 0), stop=(dc == NDC - 1))
                for pr in range(NPAIR):
                    nc.vector.tensor_scalar_mul(kT[pr][:, 0:SPAN], ka[pr], 1.0)
                va = accs4()
                for dc in range(NDC):
                    for tb in range(4):
                        nc.tensor.matmul(
                            va[tb], xt[:, dc, tb * P:(tb + 1) * P], wv[:, dc],
                            start=(dc == 0), stop=(dc == NDC - 1))
                for tb in range(4):
                    psv = va[tb].rearrange("p (pr u f) -> p pr u f", u=2, f=HD)
                    nc.vector.tensor_copy(Vb[:, tb, :, :, 0:HD], psv)

            proj0()
            carry = []
            for s in range(NSPAN):
                fillers = []
                if s + 1 < NSPAN:
                    xt = xpool.tile([P, NDC, SPAN], f16, tag="xt",
                                    name=f"xt{s + 1}")
                    xts[s + 1] = xt
                    for hdc in range(2):
                        nc.sync.dma_start(
                            xt[:, hdc * 4:(hdc + 1) * 4],
                            xT_d[:, hdc * 4:(hdc + 1) * 4,
                                 (s + 1) * SPAN:(s + 2) * SPAN])
                    pg = proj_groups(s + 1, xt)
                    if s >= 1:
                        # defer Q2/K2/Q3/K3 of the next span into that
                        # span's own filler stream (early spans have filler
                        # surplus; later spans are exp-bound and starve)
                        fillers += pg[:8]
                        carry = pg[8:]
                    else:
                        fillers += pg
                if s == 1:
                    fillers += outproj_groups(0)
                elif s == 3:
                    # span 3's own attention is ACT-bound: reserve two
                    # spans' outproj work as filler here (cpool bufs=3
                    # removes the ctxT-slot recycle deadline for span 1)
                    fillers += outproj_groups(1) + outproj_groups(2)
                attn_span(s, fillers)
            for gi in range(8):
                tb, os_ = 12 + gi // 2, gi % 2
                outproj_group(3, tb, os_, act_copy=True,
                              alt_dma=(os_ == 1))()
            if dbg:
                for i in range(NPAIR):
                    nc.sync.dma_start(dbg_h["qT_o"].ap()[i],
                                      qts[(0, i)][:].bitcast(f32))
                    nc.sync.dma_start(dbg_h["kT_o"].ap()[i], kT[i][:].bitcast(f32))
                    nc.sync.dma_start(dbg_h["ctx_o"].ap()[i][:, 3 * SPAN:],
                                      cts[(3, i)][:])
                nc.sync.dma_start(dbg_h["vb_o"].ap()[:], Vb[:])
                nc.sync.dma_start(dbg_h["id_o"].ap()[:], ident[:])
                nc.sync.dma_start(dbg_h["mask_o"].ap()[:], mask01[:])

    nc.compile()
    return nc


def get_nc():
    if "nc" not in _CACHE:
        _CACHE["nc"] = _build()
    return _CACHE["nc"]


def kernel(x, Wq, Wk, Wv, Wo, bo):
    import ml_dtypes
    from concourse import bass_utils

    x = np.asarray(x, dtype=np.float32)
    Wq, Wk, Wv = (np.asarray(w, dtype=np.float32) for w in (Wq, Wk, Wv))
    Wo = np.asarray(Wo, dtype=np.float32)
    bo = np.asarray(bo, dtype=np.float32)

    in_maps = []
    for c in range(NCORES):
        b, g = c // 2, c % 2
        gsl = slice(g * 512, (g + 1) * 512)
        in_maps.append({
            "xT": np.ascontiguousarray(x[b].T).astype(np.float16),
            "wqT": np.ascontiguousarray(Wq[gsl].T).astype(np.float16),
            "wkT": np.ascontiguousarray(Wk[gsl].T).astype(np.float16),
            "wvT": np.ascontiguousarray(Wv[gsl].T).astype(np.float16),
            "woT": np.ascontiguousarray(Wo[:, gsl].T)
            .astype(ml_dtypes.bfloat16).view(np.uint16),
        })

    nc = get_nc()
    res = bass_utils.run_bass_kernel_spmd(nc, in_maps, core_ids=list(range(NCORES)))
    parts = [res.results[c]["out"] for c in range(NCORES)]
    out = np.stack([parts[2 * b] + parts[2 * b + 1] + bo for b in range(B)])
    return out.astype(np.float32)


# revision 76
# speedup vs baseline: 1.0456x; 1.0152x over previous
"""Multi-head causal attention (B=4, T=2048, D=1024, H=16) on 8 Trainium2 cores.

Sharding: core c = (b, g) with b = c//2 (batch), g = c%2 (head-group of 8 heads).
Each core: Q/K/V projections for its 8 heads (column-parallel), causal attention,
row-parallel partial output projection. Host sums the g=0/g=1 partials + bias.

v3 design (cost-model-driven):
  - Matmul cost in the timeline model = out-free-rows x cycle x cpr, independent
    of contraction depth/partitions. fp32r: cpr=1 only for N>=256; bf16: cpr=1
    at any N.
  - Scores (S^T layout, fp32r, N=512 spans), exp -> pt in bf16.
  - AV is FLIPPED to q-partition layout: per (q-tile 128, key-chunk) matmul with
    lhsT = pt chunk (keys x 128q, bf16), rhs = V chunk [V|1] (keys x 65, bf16),
    costing 65 rows instead of streaming 512 q columns: 143k -> 71k rows.
    AV runs QTILE-MAJOR (each (qt,u) accumulation contiguous) because psum
    allows only one open accumulation group per 2KB bank; pt chunks for the
    whole pair stay buffered in SBUF.
  - ctx comes out q-major; normalize by 1/Z (psum col 64) via per-partition
    TensorScalarPtr, then PE-transpose (128x128, bf16) back to ctxT layout for
    the row-parallel output projection (bf16 x bf16, N=512).
  - qT/ctxT are span-sliced pool tiles (only one span live); kT persists full.
  - proj(s+1) and outproj(s-1) groups are spread as PE fillers through
    attention(s) so PE never stalls on the exp (ACT) chain.
"""

import os
import sys

try:
    import concourse.bass  # noqa: F401
except ImportError:  # pragma: no cover
    sys.path.insert(0, "/opt/trn_rl_repo")

import numpy as np

B, T, D = 4, 2048, 1024
H, HD = 16, 64
NCORES = 8
NPAIR = 4       # head pairs per core
NSPAN = 4       # q spans of 512
SPAN = 512
NKC = 16        # key chunks of 128
KC = 128
NDC = 8         # D chunks of 128
P = 128

_CACHE = {}


def _build():
    import concourse.bacc as bacc
    import concourse.mybir as mybir
    import concourse.tile as tile

    f32 = mybir.dt.float32
    f32r = mybir.dt.float32r
    bf16 = mybir.dt.bfloat16
    u16 = mybir.dt.uint16
    Exp = mybir.ActivationFunctionType.Exp
    Copy = mybir.ActivationFunctionType.Copy

    dbg = bool(os.environ.get("KDEBUG"))
    nc = bacc.Bacc("TRN2", target_bir_lowering=False, debug=False,
                   num_devices=1 if dbg else NCORES)

    f16 = mybir.dt.float16
    xT_h = nc.dram_tensor("xT", (D, T), f16, kind="ExternalInput")
    wqT_h = nc.dram_tensor("wqT", (D, 512), f16, kind="ExternalInput")
    wkT_h = nc.dram_tensor("wkT", (D, 512), f16, kind="ExternalInput")
    wvT_h = nc.dram_tensor("wvT", (D, 512), f16, kind="ExternalInput")
    woT_h = nc.dram_tensor("woT", (512, D), u16, kind="ExternalInput")
    out_h = nc.dram_tensor("out", (T, D), f32, kind="ExternalOutput")
    if dbg:
        dbg_h = {
            "qT_o": nc.dram_tensor("qT_o", (NPAIR, P, SPAN), f32, kind="ExternalOutput"),
            "kT_o": nc.dram_tensor("kT_o", (NPAIR, P, T), f32, kind="ExternalOutput"),
            "vb_o": nc.dram_tensor("vb_o", (P, NKC, NPAIR, 2, HD + 1), bf16,
                                   kind="ExternalOutput"),
            "ctx_o": nc.dram_tensor("ctx_o", (NPAIR, P, T), bf16,
                                    kind="ExternalOutput"),
            "pt_o": nc.dram_tensor("pt_o", (4, P, 2, SPAN), bf16,
                                   kind="ExternalOutput"),
            "av_o": nc.dram_tensor("av_o", (2, P, 4, HD + 1), f32,
                                   kind="ExternalOutput"),
            "id_o": nc.dram_tensor("id_o", (P, P), bf16, kind="ExternalOutput"),
            "mask_o": nc.dram_tensor("mask_o", (P, KC), bf16,
                                     kind="ExternalOutput"),
        }

    xT_d = xT_h.ap().rearrange("(dc p) t -> p dc t", p=P)       # (128, 8, 2048)
    wq_d = wqT_h.ap().rearrange("(dc p) f -> p dc f", p=P)      # (128, 8, 512)
    wk_d = wkT_h.ap().rearrange("(dc p) f -> p dc f", p=P)
    wv_d = wvT_h.ap().rearrange("(dc p) f -> p dc f", p=P)
    wo_d = woT_h.ap().rearrange("(pc p) f -> p pc f", p=P)      # (128, 4, 1024)

    with tile.TileContext(nc) as tc:
        with (
            tc.tile_pool(name="persist", bufs=1) as persist,
            tc.tile_pool(name="xp", bufs=2) as xpool,
            tc.tile_pool(name="qp", bufs=2) as qpool,
            tc.tile_pool(name="cp", bufs=3) as cpool,
            tc.tile_pool(name="ptp", bufs=8) as ptpool,
            tc.tile_pool(name="sbc", bufs=6) as sbcpool,
            tc.tile_pool(name="rzp", bufs=6) as rzpool,
            tc.tile_pool(name="stg", bufs=4) as stgpool,
            tc.tile_pool(name="psS", bufs=2, space="PSUM") as psS,
            tc.tile_pool(name="psAV", bufs=2, space="PSUM") as psAV,
            tc.tile_pool(name="psT", bufs=2, space="PSUM") as psTr,
        ):
            kT = [persist.tile([P, T], f16, tag=f"kT{i}", name=f"kT{i}")
                  for i in range(NPAIR)]
            # [V | 1] per (key-chunk, pair, head): ones col -> Z in AV psum col 64
            Vb = persist.tile([P, NKC, NPAIR, 2, HD + 1], bf16, tag="Vb", name="Vb")
            wq = persist.tile([P, NDC, 512], f16, tag="wq", name="wq")
            wk = persist.tile([P, NDC, 512], f16, tag="wk", name="wk")
            wv = persist.tile([P, NDC, 512], f16, tag="wv", name="wv")
            wo = persist.tile([P, 4, D], u16, tag="wo", name="wo")
            mask01 = persist.tile([P, KC], bf16, tag="mask01", name="mask01")
            ident = persist.tile([P, P], bf16, tag="ident", name="ident")
            one = nc.const_aps.tensor(1.0, (P, 1))

            nc.vector.tensor_copy(
                Vb[:, :, :, :, HD:HD + 1], one.to_broadcast((P, NKC, NPAIR, 2, 1)))
            # causal diag mask: mask01[p, f] = 1.0 if p <= f else 0.0
            nc.gpsimd.memset(mask01[:], 1.0)
            nc.gpsimd.affine_select(
                out=mask01[:], in_=mask01[:],
                compare_op=mybir.AluOpType.is_ge, fill=0.0,
                base=0, channel_multiplier=-1, pattern=[[1, KC]],
            )
            # identity for PE transpose: keep p <= f, then keep p >= f
            nc.gpsimd.memset(ident[:], 1.0)
            nc.gpsimd.affine_select(
                out=ident[:], in_=ident[:],
                compare_op=mybir.AluOpType.is_ge, fill=0.0,
                base=0, channel_multiplier=-1, pattern=[[1, P]],
            )
            nc.gpsimd.affine_select(
                out=ident[:], in_=ident[:],
                compare_op=mybir.AluOpType.is_ge, fill=0.0,
                base=0, channel_multiplier=1, pattern=[[-1, P]],
            )

            # ---- initial DMAs: wq/x0 first (Q proj starts earliest), then
            # wk (attention needs kT early), wv, wo ----
            # per-DMA queue cost is ~1.26us REGARDLESS of size (SEQ +
            # HWDGE fixed overheads; fp16 transfers are ~0.2us) -> minimize
            # DMA count: one first chunk, then two coarse tails per tensor
            xts = {0: xpool.tile([P, NDC, SPAN], f16, tag="xt", name="xt0")}
            nc.sync.dma_start(wq[:, 0:1], wq_d[:, 0:1])
            nc.scalar.dma_start(xts[0][:, 0:1], xT_d[:, 0:1, 0:SPAN])
            nc.sync.dma_start(wq[:, 1:4], wq_d[:, 1:4])
            nc.scalar.dma_start(xts[0][:, 1:4], xT_d[:, 1:4, 0:SPAN])
            nc.sync.dma_start(wq[:, 4:8], wq_d[:, 4:8])
            nc.scalar.dma_start(xts[0][:, 4:8], xT_d[:, 4:8, 0:SPAN])
            nc.sync.dma_start(wk[:, 0:4], wk_d[:, 0:4])
            nc.scalar.dma_start(wv[:, 0:4], wv_d[:, 0:4])
            nc.sync.dma_start(wk[:, 4:8], wk_d[:, 4:8])
            nc.scalar.dma_start(wv[:, 4:8], wv_d[:, 4:8])
            nc.sync.dma_start(wo[:], wo_d[:])

            qts = {}   # (sp, pr) -> (P, SPAN) f32r tile
            cts = {}   # (sp, pr) -> (P, SPAN) bf16 tile

            # ---------- emission helpers ----------
            def proj_qk(w, pr, sp, xt, scale, isq):
                def emit():
                    ps = psTr.tile([P, SPAN], f32, tag="tr", name="psqk")
                    for dc in range(NDC):
                        nc.tensor.matmul(
                            ps[:], w[:, dc, pr * P:(pr + 1) * P], xt[:, dc],
                            start=(dc == 0), stop=(dc == NDC - 1))
                    if isq:
                        dest = qpool.tile([P, SPAN], f16, tag=f"qT{pr}",
                                          name=f"qT{pr}_{sp}")
                        qts[(sp, pr)] = dest
                        nc.vector.tensor_scalar_mul(dest[:], ps[:], scale)
                    else:
                        nc.vector.tensor_scalar_mul(
                            kT[pr][:, sp * SPAN:(sp + 1) * SPAN], ps[:], scale)
                return emit

            def proj_v(sp, tb, xt):
                def emit():
                    ps = psTr.tile([P, SPAN], f32, tag="tr", name="psv")
                    for dc in range(NDC):
                        nc.tensor.matmul(
                            ps[:], xt[:, dc, tb * P:(tb + 1) * P], wv[:, dc],
                            start=(dc == 0), stop=(dc == NDC - 1))
                    kc = sp * 4 + tb
                    psv = ps[:].rearrange("p (pr u f) -> p pr u f", u=2, f=HD)
                    nc.vector.tensor_copy(Vb[:, kc, :, :, 0:HD], psv)
                return emit

            def proj_groups(sp, xt):
                gs = [proj_qk(wq, 0, sp, xt, 0.125, True),
                      proj_qk(wk, 0, sp, xt, 1.0, False)]
                gs += [proj_v(sp, tb, xt) for tb in range(4)]
                for pr in range(1, NPAIR):
                    gs.append(proj_qk(wq, pr, sp, xt, 0.125, True))
                    gs.append(proj_qk(wk, pr, sp, xt, 1.0, False))
                return gs

            def outproj_group(sp, tb, os_, alt_pool=False, act_copy=False,
                              alt_dma=False):
                def emit():
                    if alt_pool:
                        ps = psS.tile([P, 2, SPAN], f32, tag="psS",
                                      name="pso2")[:, 0, :]
                    else:
                        ps = psTr.tile([P, SPAN], f32, tag="tr", name="pso")[:]
                    for pc in range(NPAIR):
                        nc.tensor.matmul(
                            ps,
                            cts[(sp, pc)][:, (tb - sp * 4) * P:(tb - sp * 4 + 1) * P],
                            wo[:, pc, os_ * SPAN:(os_ + 1) * SPAN].bitcast(bf16),
                            start=(pc == 0), stop=(pc == NPAIR - 1))
                    stage = stgpool.tile([P, SPAN], f32, tag="st", name="stage")
                    if act_copy:
                        nc.scalar.activation(stage[:], ps, Copy)
                    else:
                        nc.vector.tensor_copy(stage[:], ps)
                    dma_q = nc.scalar if alt_dma else nc.sync
                    dma_q.dma_start(
                        out_h.ap()[tb * P:(tb + 1) * P,
                                   os_ * SPAN:(os_ + 1) * SPAN], stage[:])
                return emit

            def outproj_groups(sp):
                return [outproj_group(sp, tb, os_)
                        for tb in range(sp * 4, (sp + 1) * 4) for os_ in range(2)]

            # ---------- attention for one span ----------
            LAG = 3

            def attn_span(s, fillers):
                K = 4 * (s + 1)
                nslot = (K + LAG + 2) * NPAIR
                state = {"slot": 0, "fi": 0}

                def pace():
                    tgt = min(len(fillers),
                              len(fillers) * (state["slot"] + 1) // nslot)
                    while state["fi"] < tgt:
                        fillers[state["fi"]]()
                        state["fi"] += 1

                def tick():
                    state["slot"] += 1
                    pace()

                for pr in range(NPAIR):
                    # 2 qtiles packed per bank; accumulation via start=False
                    # onto memset-zeroed psum (one open group per bank is a
                    # hw constraint only for start=True zero-region resets)
                    av = [psAV.tile([P, 2, 2, HD + 1], f32, tag="av",
                                    name=f"av{j}") for j in range(2)]
                    for j in range(2):
                        nc.vector.memset(av[j][:], 0.0)
                    ct = cpool.tile([P, SPAN], bf16, tag=f"cT{pr}",
                                    name=f"cT{pr}_{s}")
                    cts[(s, pr)] = ct
                    pts = {}
                    deferred = []
                    pend = []
                    qt_tile = qts[(s, pr)]

                    def emit_qk(kj, qt_tile=qt_tile, pr=pr, pts=pts):
                        m = kj - 4 * s
                        sl0 = 0 if m < 0 else m * KC
                        c0 = 0 if m < 0 else m * KC
                        ss = psS.tile([P, 2, SPAN], f32, tag="psS", name="ss")
                        pt = ptpool.tile([P, 2, SPAN], bf16, tag="pt", name="pt")
                        for u in range(2):
                            lo, hi = u * HD, (u + 1) * HD
                            nc.tensor.matmul(
                                ss[:, u, sl0:],
                                kT[pr][lo:hi, kj * KC:(kj + 1) * KC],
                                qt_tile[lo:hi, sl0:],
                                start=True, stop=True)
                        nc.scalar.activation(pt[:, :, c0:], ss[:, :, c0:], Exp)
                        if m >= 0:
                            nc.vector.tensor_mul(
                                pt[:, :, c0:c0 + KC], pt[:, :, c0:c0 + KC],
                                mask01[:].rearrange("p (u f) -> p u f", u=1)
                                .to_broadcast((P, 2, KC)))
                        if dbg and s == 0 and pr == 0:
                            nc.sync.dma_start(dbg_h["pt_o"].ap()[kj], pt[:])
                        pts[kj] = pt

                    def evict(qt, av=av, pr=pr, ct=ct):
                        j, qtl = qt // 2, qt % 2
                        if dbg and s == 0 and pr == 0 and qt in (1, 3):
                            avs = stgpool.tile([P, 2 * 2 * (HD + 1)], f32,
                                               tag="st", name="avs")
                            nc.vector.tensor_copy(
                                avs[:].rearrange("p (a u f) -> p a u f",
                                                 a=2, f=HD + 1), av[j][:])
                            nc.sync.dma_start(dbg_h["av_o"].ap()[j], avs[:])
                        rz = rzpool.tile([P, 2], f32, tag="rz", name="rz")
                        sbc = sbcpool.tile([P, 2, HD], bf16, tag="sbc", name="sbc")
                        for u in range(2):
                            nc.vector.reciprocal(
                                rz[:, u:u + 1], av[j][:, qtl, u, HD:HD + 1])
                            nc.vector.tensor_scalar_mul(
                                sbc[:, u, :], av[j][:, qtl, u, 0:HD],
                                rz[:, u:u + 1])

                        def fin():
                            psx = psTr.tile([P, P], bf16, tag="tr", name="pst")
                            nc.tensor.transpose(
                                psx[:], sbc[:].rearrange("p u f -> p (u f)"),
                                ident[:])
                            nc.vector.tensor_copy(
                                ct[:, qt * P:(qt + 1) * P], psx[:])
                        deferred.append(fin)

                    def emit_av(kj, av=av, pr=pr, pts=pts):
                        m = kj - 4 * s
                        pt = pts.pop(kj)
                        for mq in range(max(0, m), 4):
                            qi = 4 * s + mq
                            j, qtl = mq // 2, mq % 2
                            for u in range(2):
                                nc.tensor.matmul(
                                    av[j][:, qtl, u, :],
                                    pt[:, u, mq * KC:(mq + 1) * KC],
                                    Vb[:, kj, pr, u, :],
                                    start=False, stop=(kj == qi),
                                    skip_group_check=True)
                        if m >= 0:
                            evict(m)

                    for kj in range(K):
                        emit_qk(kj)
                        pend.append(kj)
                        if len(pend) > LAG:
                            emit_av(pend.pop(0))
                        if len(deferred) > 3:
                            deferred.pop(0)()
                        tick()
                    while pend:
                        emit_av(pend.pop(0))
                        if len(deferred) > 3 or (not pend and deferred):
                            deferred.pop(0)()
                        tick()
                    while deferred:
                        deferred.pop(0)()
                        tick()
                # flush remaining fillers
                while state["fi"] < len(fillers):
                    fillers[state["fi"]]()
                    state["fi"] += 1

            # ---------- main schedule ----------
            # span-0 projections dc-major so matmul consumption paces with
            # chunkwise DMA arrival; 4 accumulators (2 psTr + 2 psS banks,
            # free at startup). First matmuls run half-N on the split first
            # chunks; h==0's start=True zeroes the whole psum zero-region so
            # h==1 accumulates with start=False.
            def proj0():
                def accs4():
                    a = [psTr.tile([P, SPAN], f32, tag="tr", name="p0")[:]
                         for _ in range(2)]
                    a += [psS.tile([P, 2, SPAN], f32, tag="psS",
                                   name="p0s")[:, 0, :] for _ in range(2)]
                    return a
                xt = xts[0]
                qa = accs4()
                for dc in range(NDC):
                    for pr in range(NPAIR):
                        nc.tensor.matmul(
                            qa[pr], wq[:, dc, pr * P:(pr + 1) * P], xt[:, dc],
                            start=(dc == 0), stop=(dc == NDC - 1))
                for pr in range(NPAIR):
                    dest = qpool.tile([P, SPAN], f16, tag=f"qT{pr}",
                                      name=f"qT{pr}_0")
                    qts[(0, pr)] = dest
                    nc.vector.tensor_scalar_mul(dest[:], qa[pr], 0.125)
                ka = accs4()
                for dc in range(NDC):
                    for pr in range(NPAIR):
                        nc.tensor.matmul(
                            ka[pr], wk[:, dc, pr * P:(pr + 1) * P], xt[:, dc],
                            start=(dc == 0), stop=(dc == NDC - 1))
                for pr in range(NPAIR):
                    nc.vector.tensor_scalar_mul(kT[pr][:, 0:SPAN], ka[pr], 1.0)
                va = accs4()
                for dc in range(NDC):
                    for tb in range(4):
                        nc.tensor.matmul(
                            va[tb], xt[:, dc, tb * P:(tb + 1) * P], wv[:, dc],
                            start=(dc == 0), stop=(dc == NDC - 1))
                for tb in range(4):
                    psv = va[tb].rearrange("p (pr u f) -> p pr u f", u=2, f=HD)
                    nc.vector.tensor_copy(Vb[:, tb, :, :, 0:HD], psv)

            proj0()
            carry = []
            for s in range(NSPAN):
                fillers = []
                if s + 1 < NSPAN:
                    xt = xpool.tile([P, NDC, SPAN], f16, tag="xt",
                                    name=f"xt{s + 1}")
                    xts[s + 1] = xt
                    for hdc in range(2):
                        nc.sync.dma_start(
                            xt[:, hdc * 4:(hdc + 1) * 4],
                            xT_d[:, hdc * 4:(hdc + 1) * 4,
                                 (s + 1) * SPAN:(s + 2) * SPAN])
                    pg = proj_groups(s + 1, xt)
                    if s >= 1:
                        # defer Q2/K2/Q3/K3 of the next span into that
                        # span's own filler stream (early spans have filler
                        # surplus; later spans are exp-bound and starve)
                        fillers += pg[:8]
                        carry = pg[8:]
                    else:
                        fillers += pg
                if s == 1:
                    fillers += outproj_groups(0)
                elif s == 3:
                    # span 3's own attention is ACT-bound: reserve two
                    # spans' outproj work as filler here (cpool bufs=3
                    # removes the ctxT-slot recycle deadline for span 1)
                    fillers += outproj_groups(1) + outproj_groups(2)
                attn_span(s, fillers)
            for gi in range(8):
                tb, os_ = 12 + gi // 2, gi % 2
                outproj_group(3, tb, os_, act_copy=True,
                              alt_dma=(os_ == 1))()
            if dbg:
                for i in range(NPAIR):
                    nc.sync.dma_start(dbg_h["qT_o"].ap()[i],
                                      qts[(0, i)][:].bitcast(f32))
                    nc.sync.dma_start(dbg_h["kT_o"].ap()[i], kT[i][:].bitcast(f32))
                    nc.sync.dma_start(dbg_h["ctx_o"].ap()[i][:, 3 * SPAN:],
                                      cts[(3, i)][:])
                nc.sync.dma_start(dbg_h["vb_o"].ap()[:], Vb[:])
                nc.sync.dma_start(dbg_h["id_o"].ap()[:], ident[:])
                nc.sync.dma_start(dbg_h["mask_o"].ap()[:], mask01[:])

    nc.compile()
    return nc


def get_nc():
    if "nc" not in _CACHE:
        _CACHE["nc"] = _build()
    return _CACHE["nc"]


def kernel(x, Wq, Wk, Wv, Wo, bo):
    import ml_dtypes
    from concourse import bass_utils

    x = np.asarray(x, dtype=np.float32)
    Wq, Wk, Wv = (np.asarray(w, dtype=np.float32) for w in (Wq, Wk, Wv))
    Wo = np.asarray(Wo, dtype=np.float32)
    bo = np.asarray(bo, dtype=np.float32)

    in_maps = []
    for c in range(NCORES):
        b, g = c // 2, c % 2
        gsl = slice(g * 512, (g + 1) * 512)
        in_maps.append({
            "xT": np.ascontiguousarray(x[b].T).astype(np.float16),
            "wqT": np.ascontiguousarray(Wq[gsl].T).astype(np.float16),
            "wkT": np.ascontiguousarray(Wk[gsl].T).astype(np.float16),
            "wvT": np.ascontiguousarray(Wv[gsl].T).astype(np.float16),
            "woT": np.ascontiguousarray(Wo[:, gsl].T)
            .astype(ml_dtypes.bfloat16).view(np.uint16),
        })

    nc = get_nc()
    res = bass_utils.run_bass_kernel_spmd(nc, in_maps, core_ids=list(range(NCORES)))
    parts = [res.results[c]["out"] for c in range(NCORES)]
    out = np.stack([parts[2 * b] + parts[2 * b + 1] + bo for b in range(B)])
    return out.astype(np.float32)
